# revision 22
# baseline (speedup 1.0000x reference)
"""Trainium2 Bass kernel for the HARNN local decoder problem.

Mathematical structure exploited (verified to ~5e-6 abs err vs the fp32
reference):
  - In text_attention, att_out = einsum('bsc,bsd->bd', softmax_w, x): the
    sum over c of a softmax is exactly 1, so att_out = sum_s x[b,s,:].
  - In local_layer, visual = mean_c(softmax_c(...)) = 1/C exactly, so the
    per-level attention recomputation never affects the output.
  Hence:
    att    = sum_s lstm_out[b,s,:]                      # [B, D]
    li     = concat([lstm_pool, att], -1)               # [B, 2D]
    out_h  = sigmoid(relu(li @ w_fc[h] + b_fc[h]) @ w_lc[h] + b_lc[h])
    out    = concat(out_h, axis=1)                      # [B, 2798]

Distribution: pure data parallel over batch (32 = 8 cores x 4 samples);
weights replicated per core.  (Collectives on this runtime have a ~70us
fixed cost, so weight-sharded variants lose.)

PE dtype: float32r (single-pass fp32 matmul, ~3.2x faster than the
LOW_HIGH two-pass fp32 mode; measured final output abs err 2.8e-3 on
sigmoid outputs with absmax 1.0, vs 4.5e-6 all-fp32).  The S-reduction
runs on the vector engine (exact fp32), with a single fp32r ones-matmul
per sample for the partition reduce.  The DMA stream (17.9MB/core: 4MB
lstm shard + 13.9MB weights) runs at the ~358GB/s HBM-per-core limit
and is the roofline; measured HW exec ~70-74us/core (vs ~100us for the
all-fp32 variant of the same structure).
"""

import numpy as np

import concourse.bass as bass
import concourse.bacc as bacc
import concourse.mybir as mybir
import concourse.tile as tile
from concourse.bass_utils import run_bass_kernel_spmd


def _ensure_ntff_hook_module():
    """bass_utils imports antenv.axon_hooks when tracing is requested;
    the bare container's antenv stub lacks it.  Provide the ctypes-based
    hook from trn_boot when available, else a no-op, so BASS_TRACE=1 in
    the environment cannot crash kernel()."""
    import sys
    import types
    try:
        import antenv
        import antenv.axon_hooks  # noqa: F401
        return
    except ImportError:
        pass
    hook = None
    try:
        sys.path.insert(0, "/root/.axon_site")
        from trn_agent_boot.trn_boot import _ntff_profile_via_ctypes
        hook = _ntff_profile_via_ctypes("/opt/axon/libaxon_pjrt.so")
    except Exception:
        hook = None
    mod = types.ModuleType("antenv.axon_hooks")
    mod.get_axon_ntff_profile_hook = lambda: hook
    mod.set_axon_ntff_profile_hook = lambda h: None
    sys.modules["antenv.axon_hooks"] = mod
    try:
        antenv.axon_hooks = mod
    except Exception:
        pass


_ensure_ntff_hook_module()

F32 = mybir.dt.float32
F32R = mybir.dt.float32r

B = 32
NCORES = 8
BL = B // NCORES          # 4 samples per core
D = 512
FC = 512
S = 512
HIER = [9, 128, 661, 2000]
HIERP = [16, 128, 664, 2000]     # padded to x8 for fp32r matmul N restriction
CTOT = sum(HIER)          # 2798
CTOTP = sum(HIERP)        # 2808
AF = mybir.ActivationFunctionType

_NC = {}
LAST_RESULTS = None


def _chunks(total, step):
    out = []
    c0 = 0
    while c0 < total:
        out.append((c0, min(step, total - c0)))
        c0 += step
    return out


def _build(use_bias):
    nc = bacc.Bacc("TRN2", target_bir_lowering=False, debug=False,
                   num_devices=NCORES)

    x_d = nc.dram_tensor("x", [128, BL, 4, D], F32R, kind="ExternalInput")
    poolT_d = nc.dram_tensor("poolT", [D, BL], F32R, kind="ExternalInput")
    wfc_d = nc.dram_tensor("wfc", [128, 4, 8, FC], F32R,
                           kind="ExternalInput")
    bfc_d = nc.dram_tensor("bfc", [1, 4, FC], F32R, kind="ExternalInput")
    wlc_d = [nc.dram_tensor(f"wlc{h}", [128, 4, HIERP[h]], F32R,
                            kind="ExternalInput") for h in range(3)]
    wlc_d.append(nc.dram_tensor("wlc3", [128, 4, 4, 500], F32R,
                                kind="ExternalInput"))
    blc_d = nc.dram_tensor("blc", [1, CTOTP + BL], F32R,
                           kind="ExternalInput")
    aux_d = nc.dram_tensor("aux", [128, BL * BL + BL], F32R,
                           kind="ExternalInput")
    out_d = nc.dram_tensor("out", [BL, CTOTP], F32, kind="ExternalOutput")

    with tile.TileContext(nc) as tc:
        with (
            tc.tile_pool(name="sb", bufs=1) as sb,
            tc.tile_pool(name="ps_att", bufs=1, space="PSUM") as ps_att_pool,
            tc.tile_pool(name="ps_mm", bufs=2, space="PSUM") as ps_mm_pool,
            tc.tile_pool(name="ps_t", bufs=2, space="PSUM") as ps_t_pool,
        ):
            # ---- constants / small inputs ----
            # aux packs sel [128,16] and eye4 (rows 0-3 of cols 16:20)
            aux = sb.tile([128, BL * BL + BL], F32R)
            nc.sync.dma_start(aux[:], aux_d[:])
            sel = aux[:, 0:BL * BL]
            eye4 = aux[0:BL, BL * BL:BL * BL + BL]
            bfc_sb = sb.tile([1, 4, FC], F32R)
            blc_sb = sb.tile([1, CTOTP + BL], F32R)
            if use_bias:
                nc.sync.dma_start(bfc_sb[:], bfc_d[:])
                nc.sync.dma_start(blc_sb[:], blc_d[:])
            ones_row = blc_sb[:, CTOTP:CTOTP + BL]


            # liT: [128, kt(8), b(4)]  (local_input transposed, 2D x BL)
            liT = sb.tile([128, 8, BL], F32R)
            nc.sync.dma_start(
                liT[:, 0:4, :],
                poolT_d[:].rearrange("(t p) b -> p t b", p=128),
            )

            # ---- lstm shard + DVE rowsum (exact fp32 adds) ----
            x_sb = sb.tile([128, BL, 4, D], F32R)
            part = sb.tile([128, BL, D], F32R)
            for b in range(BL):
                nc.sync.dma_start(x_sb[:, b], x_d[:, b])
                nc.vector.tensor_add(part[:, b], x_sb[:, b, 0, :],
                                     x_sb[:, b, 1, :])
                nc.vector.tensor_add(part[:, b], part[:, b], x_sb[:, b, 2, :])
                nc.vector.tensor_add(part[:, b], part[:, b], x_sb[:, b, 3, :])

            # ---- weights, in consumption order ----
            wfc_sb = sb.tile([128, 4, 8, FC], F32R)
            wlc_sb = []
            for h in range(4):
                nc.sync.dma_start(wfc_sb[:, h], wfc_d[:, h])
                wt = sb.tile([128, 4, HIERP[h]], F32R, tag=f"wlc{h}")
                if h < 3:
                    nc.sync.dma_start(wt[:], wlc_d[h][:])
                else:
                    for ch in range(4):
                        nc.sync.dma_start(wt[:, :, 500 * ch:500 * (ch + 1)],
                                          wlc_d[3][:, ch])
                wlc_sb.append(wt)

            # ---- partition reduce: att[b, :] = colsum(part[:, b]) ----
            psum_att = ps_att_pool.tile([BL, D], F32)
            for b in range(BL):
                nc.tensor.matmul(
                    psum_att[:], sel[:, BL * b:BL * (b + 1)], part[:, b],
                    start=(b == 0), stop=(b == 3),
                )
            att_sb = sb.tile([BL, D], F32R)
            nc.vector.tensor_copy(att_sb[:], psum_att[:])

            # transpose att [4, 512] -> liT[:, 4+c, :]
            for c in range(4):
                pt = ps_t_pool.tile([128, BL], F32R, tag="pt")
                nc.tensor.transpose(pt[:], att_sb[:, 128 * c:128 * (c + 1)],
                                    eye4)
                nc.vector.tensor_copy(liT[:, 4 + c, :], pt[:])

            out_sb = sb.tile([BL, CTOTP], F32)

            # ---- per-level MLP ----
            off = 0
            for h in range(4):
                Ch = HIERP[h]
                # fc = relu(li @ w_fc[h] + b_fc[h])  -> psum [4, 512]
                psum_fc = ps_mm_pool.tile([BL, FC], F32, tag="fc")
                for kt in range(8):
                    nc.tensor.matmul(psum_fc[:], liT[:, kt, :],
                                     wfc_sb[:, h, kt, :],
                                     start=(kt == 0),
                                     stop=(not use_bias and kt == 7))
                if use_bias:
                    nc.tensor.matmul(psum_fc[:], ones_row, bfc_sb[:, h, :],
                                     start=False, stop=True)
                fc_sb = sb.tile([BL, FC], F32R, tag="fc_sb")
                nc.scalar.activation(fc_sb[:], psum_fc[:], AF.Relu)

                # fcT [128, kt(4), b]
                fcT = sb.tile([128, 4, BL], F32R, tag="fcT")
                for c in range(4):
                    pt = ps_t_pool.tile([128, BL], F32R, tag="pt")
                    nc.tensor.transpose(pt[:], fc_sb[:, 128 * c:128 * (c + 1)],
                                        eye4)
                    nc.vector.tensor_copy(fcT[:, c, :], pt[:])

                # scores = sigmoid(fc @ w_lc[h] + b_lc[h])
                for (c0, n) in _chunks(Ch, 512):
                    ps = ps_mm_pool.tile([BL, 512], F32, tag="ps")
                    for kt in range(4):
                        nc.tensor.matmul(ps[:, :n], fcT[:, kt, :],
                                         wlc_sb[h][:, kt, c0:c0 + n],
                                         start=(kt == 0),
                                         stop=(not use_bias and kt == 3))
                    if use_bias:
                        nc.tensor.matmul(ps[:, :n], ones_row,
                                         blc_sb[:, off + c0:off + c0 + n],
                                         start=False, stop=True)
                    nc.scalar.activation(out_sb[:, off + c0:off + c0 + n],
                                         ps[:, :n], AF.Sigmoid)
                off += Ch

            nc.sync.dma_start(out_d[:], out_sb[:])

    nc.compile()
    return nc


def kernel(lstm_out, lstm_pool, w_at1, w_at2, w_fc, b_fc, w_lc, b_lc):
    global _NC, LAST_RESULTS
    use_bias = any(np.any(np.asarray(b) != 0) for b in list(b_fc) + list(b_lc))
    if use_bias not in _NC:
        _NC[use_bias] = _build(use_bias)
    nc = _NC[use_bias]

    lstm_out = np.asarray(lstm_out, dtype=np.float32)
    lstm_pool = np.asarray(lstm_pool, dtype=np.float32)
    # [4, 1024, 512] -> [128(p), 4(h), 8(t), 512]
    wfc_np = np.ascontiguousarray(
        np.stack([np.asarray(w, np.float32) for w in w_fc])
        .reshape(4, 8, 128, FC).transpose(2, 0, 1, 3))
    bfc_np = np.ascontiguousarray(np.stack([np.asarray(b, np.float32) for b in b_fc])[None])
    wlc_np = []
    blc_parts = []
    for h in range(4):
        wp = np.zeros([FC, HIERP[h]], np.float32)
        wp[:, :HIER[h]] = np.asarray(w_lc[h], np.float32)
        # [512, CP] -> [128(p), 4(t), CP]
        wpp = wp.reshape(4, 128, HIERP[h]).transpose(1, 0, 2)
        if h < 3:
            wlc_np.append(np.ascontiguousarray(wpp))
        else:
            # [128, 4(t), 2000] -> [128, 4(chunk), 4(t), 500]
            wlc_np.append(np.ascontiguousarray(
                wpp.reshape(128, 4, 4, 500).transpose(0, 2, 1, 3)))
        bp = np.zeros([HIERP[h]], np.float32)
        bp[:HIER[h]] = np.asarray(b_lc[h], np.float32)
        blc_parts.append(bp)
    blc_np = np.ascontiguousarray(np.concatenate(blc_parts)[None])
    aux = np.zeros([128, BL * BL + BL], np.float32)
    for b in range(BL):
        aux[:, BL * b + b] = 1.0
    aux[0:BL, BL * BL:BL * BL + BL] = np.eye(BL, dtype=np.float32)
    blc_np = np.ascontiguousarray(
        np.concatenate([blc_np[0], np.ones([BL], np.float32)])[None])

    in_maps = []
    for k in range(NCORES):
        m = {
            "x": np.ascontiguousarray(
                lstm_out[BL * k: BL * (k + 1)]
                .reshape(BL, 4, 128, D).transpose(2, 0, 1, 3)),
            "poolT": np.ascontiguousarray(lstm_pool[BL * k: BL * (k + 1)].T),
            "wfc": wfc_np,
            "bfc": bfc_np,
            "blc": blc_np,
            "aux": aux,
        }
        for h in range(4):
            m[f"wlc{h}"] = wlc_np[h]
        in_maps.append(m)

    expected = set()
    for alloc in nc.m.functions[0].allocations:
        if isinstance(alloc, mybir.MemoryLocationSet) and \
                alloc.kind == "ExternalInput":
            expected.add(alloc.memorylocations[0].name)
    in_maps = [{k: v for k, v in m.items() if k in expected}
               for m in in_maps]

    res = run_bass_kernel_spmd(nc, in_maps, list(range(NCORES)))
    LAST_RESULTS = res
    full = np.concatenate([r["out"] for r in res.results], axis=0)
    # trim per-level fp32r padding columns
    outs = []
    off = 0
    for h in range(4):
        outs.append(full[:, off:off + HIER[h]])
        off += HIERP[h]
    return np.ascontiguousarray(np.concatenate(outs, axis=1))


# revision 24
# speedup vs baseline: 1.0308x; 1.0308x over previous
"""Trainium2 Bass kernel for the HARNN local decoder problem.

Mathematical structure exploited (verified to ~5e-6 abs err vs the fp32
reference):
  - In text_attention, att_out = einsum('bsc,bsd->bd', softmax_w, x): the
    sum over c of a softmax is exactly 1, so att_out = sum_s x[b,s,:].
  - In local_layer, visual = mean_c(softmax_c(...)) = 1/C exactly, so the
    per-level attention recomputation never affects the output.
  Hence:
    att    = sum_s lstm_out[b,s,:]                      # [B, D]
    li     = concat([lstm_pool, att], -1)               # [B, 2D]
    out_h  = sigmoid(relu(li @ w_fc[h] + b_fc[h]) @ w_lc[h] + b_lc[h])
    out    = concat(out_h, axis=1)                      # [B, 2798]

Distribution: pure data parallel over batch (32 = 8 cores x 4 samples);
weights replicated per core.  (Collectives on this runtime have a ~70us
fixed cost, so weight-sharded variants lose.)

PE dtype: float32r (single-pass fp32 matmul, ~3.2x faster than the
LOW_HIGH two-pass fp32 mode; measured final output abs err 2.8e-3 on
sigmoid outputs with absmax 1.0, vs 4.5e-6 all-fp32).  The S-reduction
runs on the vector engine (exact fp32), with a single fp32r ones-matmul
per sample for the partition reduce.  The DMA stream (17.9MB/core: 4MB
lstm shard + 13.9MB weights) runs at the ~358GB/s HBM-per-core limit
and is the roofline; measured HW exec ~70-74us/core (vs ~100us for the
all-fp32 variant of the same structure).
"""

import numpy as np

import concourse.bass as bass
import concourse.bacc as bacc
import concourse.mybir as mybir
import concourse.tile as tile
from concourse.bass_utils import run_bass_kernel_spmd


def _ensure_ntff_hook_module():
    """bass_utils imports antenv.axon_hooks when tracing is requested;
    the bare container's antenv stub lacks it.  Provide the ctypes-based
    hook from trn_boot when available, else a no-op, so BASS_TRACE=1 in
    the environment cannot crash kernel()."""
    import sys
    import types
    try:
        import antenv
        import antenv.axon_hooks  # noqa: F401
        return
    except ImportError:
        pass
    hook = None
    try:
        sys.path.insert(0, "/root/.axon_site")
        from trn_agent_boot.trn_boot import _ntff_profile_via_ctypes
        hook = _ntff_profile_via_ctypes("/opt/axon/libaxon_pjrt.so")
    except Exception:
        hook = None
    mod = types.ModuleType("antenv.axon_hooks")
    mod.get_axon_ntff_profile_hook = lambda: hook
    mod.set_axon_ntff_profile_hook = lambda h: None
    sys.modules["antenv.axon_hooks"] = mod
    try:
        antenv.axon_hooks = mod
    except Exception:
        pass


_ensure_ntff_hook_module()

F32 = mybir.dt.float32
F32R = mybir.dt.float32r

B = 32
NCORES = 8
BL = B // NCORES          # 4 samples per core
D = 512
FC = 512
S = 512
HIER = [9, 128, 661, 2000]
HIERP = [16, 128, 664, 2000]     # padded to x8 for fp32r matmul N restriction
CTOT = sum(HIER)          # 2798
CTOTP = sum(HIERP)        # 2808
AF = mybir.ActivationFunctionType

_NC = {}
LAST_RESULTS = None


def _chunks(total, step):
    out = []
    c0 = 0
    while c0 < total:
        out.append((c0, min(step, total - c0)))
        c0 += step
    return out


# level-3 column chunks: keep the LAST chunk tiny so the compute tail
# after the final weight byte lands is minimal
CHUNKS3 = [(0, 512), (512, 512), (1024, 512), (1536, 400), (1936, 64)]


def _build(use_bias):
    nc = bacc.Bacc("TRN2", target_bir_lowering=False, debug=False,
                   num_devices=NCORES)

    x_d = nc.dram_tensor("x", [BL, S, D], F32R, kind="ExternalInput")
    poolT_d = nc.dram_tensor("poolT", [D, BL], F32R, kind="ExternalInput")
    wfc_d = nc.dram_tensor("wfc", [4, 2 * D, FC], F32R, kind="ExternalInput")
    bfc_d = nc.dram_tensor("bfc", [1, 4, FC], F32R, kind="ExternalInput")
    wlc_d = [nc.dram_tensor(f"wlc{h}", [FC, HIERP[h]], F32R,
                            kind="ExternalInput") for h in range(4)]
    blc_d = nc.dram_tensor("blc", [1, CTOTP + BL], F32R,
                           kind="ExternalInput")
    aux_d = nc.dram_tensor("aux", [128, BL * BL + BL], F32R,
                           kind="ExternalInput")
    out_d = nc.dram_tensor("out", [BL, CTOTP], F32, kind="ExternalOutput")

    with tile.TileContext(nc) as tc:
        with (
            tc.tile_pool(name="sb", bufs=1) as sb,
            tc.tile_pool(name="ps_att", bufs=1, space="PSUM") as ps_att_pool,
            tc.tile_pool(name="ps_mm", bufs=2, space="PSUM") as ps_mm_pool,
            tc.tile_pool(name="ps_t", bufs=2, space="PSUM") as ps_t_pool,
        ):
            # ---- constants / small inputs ----
            # aux packs sel [128,16] and eye4 (rows 0-3 of cols 16:20)
            aux = sb.tile([128, BL * BL + BL], F32R)
            nc.sync.dma_start(aux[:], aux_d[:])
            sel = aux[:, 0:BL * BL]
            eye4 = aux[0:BL, BL * BL:BL * BL + BL]
            bfc_sb = sb.tile([1, 4, FC], F32R)
            blc_sb = sb.tile([1, CTOTP + BL], F32R)
            if use_bias:
                nc.sync.dma_start(bfc_sb[:], bfc_d[:])
                nc.sync.dma_start(blc_sb[:], blc_d[:])
            ones_row = blc_sb[:, CTOTP:CTOTP + BL]


            # liT: [128, kt(8), b(4)]  (local_input transposed, 2D x BL)
            liT = sb.tile([128, 8, BL], F32R)
            nc.sync.dma_start(
                liT[:, 0:4, :],
                poolT_d[:].rearrange("(t p) b -> p t b", p=128),
            )

            # ---- lstm shard + DVE rowsum (exact fp32 adds) ----
            x_sb = sb.tile([128, BL, 4, D], F32R)
            part = sb.tile([128, BL, D], F32R)
            for b in range(BL):
                nc.sync.dma_start(
                    x_sb[:, b], x_d[b].rearrange("(t p) d -> p t d", p=128)
                )
                nc.vector.tensor_add(part[:, b], x_sb[:, b, 0, :],
                                     x_sb[:, b, 1, :])
                nc.vector.tensor_add(part[:, b], part[:, b], x_sb[:, b, 2, :])
                nc.vector.tensor_add(part[:, b], part[:, b], x_sb[:, b, 3, :])

            # ---- weights, in consumption order ----
            wfc_sb = sb.tile([128, 4, 8, FC], F32R)
            wlc_sb = []
            for h in range(4):
                nc.sync.dma_start(
                    wfc_sb[:, h], wfc_d[h].rearrange("(t p) n -> p t n", p=128)
                )
                wt = sb.tile([128, 4, HIERP[h]], F32R, tag=f"wlc{h}")
                ch = CHUNKS3 if h == 3 else _chunks(HIERP[h], 512)
                for (c0, n) in ch:
                    nc.sync.dma_start(
                        wt[:, :, c0:c0 + n],
                        wlc_d[h][:, c0:c0 + n].rearrange(
                            "(t p) c -> p t c", p=128))
                wlc_sb.append(wt)

            # ---- partition reduce: att[b, :] = colsum(part[:, b]) ----
            psum_att = ps_att_pool.tile([BL, D], F32)
            for b in range(BL):
                nc.tensor.matmul(
                    psum_att[:], sel[:, BL * b:BL * (b + 1)], part[:, b],
                    start=(b == 0), stop=(b == 3),
                )
            att_sb = sb.tile([BL, D], F32R)
            nc.vector.tensor_copy(att_sb[:], psum_att[:])

            # transpose att [4, 512] -> liT[:, 4+c, :]
            for c in range(4):
                pt = ps_t_pool.tile([128, BL], F32R, tag="pt")
                nc.tensor.transpose(pt[:], att_sb[:, 128 * c:128 * (c + 1)],
                                    eye4)
                nc.vector.tensor_copy(liT[:, 4 + c, :], pt[:])

            out_sb = sb.tile([BL, CTOTP], F32)

            # ---- per-level MLP ----
            off = 0
            for h in range(4):
                Ch = HIERP[h]
                # fc = relu(li @ w_fc[h] + b_fc[h])  -> psum [4, 512]
                psum_fc = ps_mm_pool.tile([BL, FC], F32, tag="fc")
                for kt in range(8):
                    nc.tensor.matmul(psum_fc[:], liT[:, kt, :],
                                     wfc_sb[:, h, kt, :],
                                     start=(kt == 0),
                                     stop=(not use_bias and kt == 7))
                if use_bias:
                    nc.tensor.matmul(psum_fc[:], ones_row, bfc_sb[:, h, :],
                                     start=False, stop=True)
                fc_sb = sb.tile([BL, FC], F32R, tag="fc_sb")
                nc.scalar.activation(fc_sb[:], psum_fc[:], AF.Relu)

                # fcT [128, kt(4), b]
                fcT = sb.tile([128, 4, BL], F32R, tag="fcT")
                for c in range(4):
                    pt = ps_t_pool.tile([128, BL], F32R, tag="pt")
                    nc.tensor.transpose(pt[:], fc_sb[:, 128 * c:128 * (c + 1)],
                                        eye4)
                    nc.vector.tensor_copy(fcT[:, c, :], pt[:])

                # scores = sigmoid(fc @ w_lc[h] + b_lc[h])
                score_chunks = CHUNKS3 if h == 3 else _chunks(Ch, 512)
                for (c0, n) in score_chunks:
                    ps = ps_mm_pool.tile([BL, 512], F32, tag="ps")
                    for kt in range(4):
                        nc.tensor.matmul(ps[:, :n], fcT[:, kt, :],
                                         wlc_sb[h][:, kt, c0:c0 + n],
                                         start=(kt == 0),
                                         stop=(not use_bias and kt == 3))
                    if use_bias:
                        nc.tensor.matmul(ps[:, :n], ones_row,
                                         blc_sb[:, off + c0:off + c0 + n],
                                         start=False, stop=True)
                    nc.scalar.activation(out_sb[:, off + c0:off + c0 + n],
                                         ps[:, :n], AF.Sigmoid)
                off += Ch

            nc.sync.dma_start(out_d[:, :CTOTP - 64],
                              out_sb[:, :CTOTP - 64])
            nc.sync.dma_start(out_d[:, CTOTP - 64:],
                              out_sb[:, CTOTP - 64:])

    nc.compile()
    return nc


def kernel(lstm_out, lstm_pool, w_at1, w_at2, w_fc, b_fc, w_lc, b_lc):
    global _NC, LAST_RESULTS
    use_bias = any(np.any(np.asarray(b) != 0) for b in list(b_fc) + list(b_lc))
    if use_bias not in _NC:
        _NC[use_bias] = _build(use_bias)
    nc = _NC[use_bias]

    lstm_out = np.ascontiguousarray(np.asarray(lstm_out, dtype=np.float32))
    lstm_pool = np.asarray(lstm_pool, dtype=np.float32)
    wfc_np = np.ascontiguousarray(np.stack([np.asarray(w, np.float32) for w in w_fc]))
    bfc_np = np.ascontiguousarray(np.stack([np.asarray(b, np.float32) for b in b_fc])[None])
    wlc_np = []
    blc_parts = []
    for h in range(4):
        wp = np.zeros([FC, HIERP[h]], np.float32)
        wp[:, :HIER[h]] = np.asarray(w_lc[h], np.float32)
        wlc_np.append(wp)
        bp = np.zeros([HIERP[h]], np.float32)
        bp[:HIER[h]] = np.asarray(b_lc[h], np.float32)
        blc_parts.append(bp)
    blc_np = np.ascontiguousarray(np.concatenate(blc_parts)[None])
    aux = np.zeros([128, BL * BL + BL], np.float32)
    for b in range(BL):
        aux[:, BL * b + b] = 1.0
    aux[0:BL, BL * BL:BL * BL + BL] = np.eye(BL, dtype=np.float32)
    blc_np = np.ascontiguousarray(
        np.concatenate([blc_np[0], np.ones([BL], np.float32)])[None])

    in_maps = []
    for k in range(NCORES):
        m = {
            "x": np.ascontiguousarray(lstm_out[BL * k: BL * (k + 1)]),
            "poolT": np.ascontiguousarray(lstm_pool[BL * k: BL * (k + 1)].T),
            "wfc": wfc_np,
            "bfc": bfc_np,
            "blc": blc_np,
            "aux": aux,
        }
        for h in range(4):
            m[f"wlc{h}"] = wlc_np[h]
        in_maps.append(m)

    expected = set()
    for alloc in nc.m.functions[0].allocations:
        if isinstance(alloc, mybir.MemoryLocationSet) and \
                alloc.kind == "ExternalInput":
            expected.add(alloc.memorylocations[0].name)
    in_maps = [{k: v for k, v in m.items() if k in expected}
               for m in in_maps]

    res = run_bass_kernel_spmd(nc, in_maps, list(range(NCORES)))
    LAST_RESULTS = res
    full = np.concatenate([r["out"] for r in res.results], axis=0)
    # trim per-level fp32r padding columns
    outs = []
    off = 0
    for h in range(4):
        outs.append(full[:, off:off + HIER[h]])
        off += HIERP[h]
    return np.ascontiguousarray(np.concatenate(outs, axis=1))


# revision 25
# speedup vs baseline: 1.0959x; 1.0632x over previous
"""Trainium2 Bass kernel for the HARNN local decoder problem.

Mathematical structure exploited (verified to ~5e-6 abs err vs the fp32
reference):
  - In text_attention, att_out = einsum('bsc,bsd->bd', softmax_w, x): the
    sum over c of a softmax is exactly 1, so att_out = sum_s x[b,s,:].
  - In local_layer, visual = mean_c(softmax_c(...)) = 1/C exactly, so the
    per-level attention recomputation never affects the output.
  Hence:
    att    = sum_s lstm_out[b,s,:]                      # [B, D]
    li     = concat([lstm_pool, att], -1)               # [B, 2D]
    out_h  = sigmoid(relu(li @ w_fc[h] + b_fc[h]) @ w_lc[h] + b_lc[h])
    out    = concat(out_h, axis=1)                      # [B, 2798]

Distribution: pure data parallel over batch (32 = 8 cores x 4 samples);
weights replicated per core.  (Collectives on this runtime have a ~70us
fixed cost, so weight-sharded variants lose.)

PE dtype: float32r (single-pass fp32 matmul, ~3.2x faster than the
LOW_HIGH two-pass fp32 mode; measured final output abs err 2.8e-3 on
sigmoid outputs with absmax 1.0, vs 4.5e-6 all-fp32).  The S-reduction
runs on the vector engine (exact fp32), with a single fp32r ones-matmul
per sample for the partition reduce.  The DMA stream (17.9MB/core: 4MB
lstm shard + 13.9MB weights) runs at the ~358GB/s HBM-per-core limit
and is the roofline; the final w_lc[3] column chunk is kept tiny (64
cols) and the output DMA split so only ~1.5us of compute trails the
last weight byte.  Measured HW exec 68-74us/core (vs ~100us for the
all-fp32 variant of the same structure).
"""

import numpy as np

import concourse.bass as bass
import concourse.bacc as bacc
import concourse.mybir as mybir
import concourse.tile as tile
from concourse.bass_utils import run_bass_kernel_spmd


def _ensure_ntff_hook_module():
    """bass_utils imports antenv.axon_hooks when tracing is requested;
    the bare container's antenv stub lacks it.  Provide the ctypes-based
    hook from trn_boot when available, else a no-op, so BASS_TRACE=1 in
    the environment cannot crash kernel()."""
    import sys
    import types
    try:
        import antenv
        import antenv.axon_hooks  # noqa: F401
        return
    except ImportError:
        pass
    hook = None
    try:
        sys.path.insert(0, "/root/.axon_site")
        from trn_agent_boot.trn_boot import _ntff_profile_via_ctypes
        hook = _ntff_profile_via_ctypes("/opt/axon/libaxon_pjrt.so")
    except Exception:
        hook = None
    mod = types.ModuleType("antenv.axon_hooks")
    mod.get_axon_ntff_profile_hook = lambda: hook
    mod.set_axon_ntff_profile_hook = lambda h: None
    sys.modules["antenv.axon_hooks"] = mod
    try:
        antenv.axon_hooks = mod
    except Exception:
        pass


_ensure_ntff_hook_module()

F32 = mybir.dt.float32
F32R = mybir.dt.float32r

B = 32
NCORES = 8
BL = B // NCORES          # 4 samples per core
D = 512
FC = 512
S = 512
HIER = [9, 128, 661, 2000]
HIERP = [16, 128, 664, 2000]     # padded to x8 for fp32r matmul N restriction
CTOT = sum(HIER)          # 2798
CTOTP = sum(HIERP)        # 2808
AF = mybir.ActivationFunctionType

_NC = {}
LAST_RESULTS = None


def _chunks(total, step):
    out = []
    c0 = 0
    while c0 < total:
        out.append((c0, min(step, total - c0)))
        c0 += step
    return out


# level-3 column chunks: keep the LAST chunk tiny so the compute tail
# after the final weight byte lands is minimal
CHUNKS3 = [(0, 512), (512, 512), (1024, 512), (1536, 400), (1936, 64)]


def _build(use_bias):
    nc = bacc.Bacc("TRN2", target_bir_lowering=False, debug=False,
                   num_devices=NCORES)

    x_d = nc.dram_tensor("x", [BL, S, D], F32R, kind="ExternalInput")
    poolT_d = nc.dram_tensor("poolT", [D, BL], F32R, kind="ExternalInput")
    wfc_d = nc.dram_tensor("wfc", [4, 2 * D, FC], F32R, kind="ExternalInput")
    bfc_d = nc.dram_tensor("bfc", [1, 4, FC], F32R, kind="ExternalInput")
    wlc_d = [nc.dram_tensor(f"wlc{h}", [FC, HIERP[h]], F32R,
                            kind="ExternalInput") for h in range(4)]
    blc_d = nc.dram_tensor("blc", [1, CTOTP + BL], F32R,
                           kind="ExternalInput")
    aux_d = nc.dram_tensor("aux", [128, BL * BL + BL], F32R,
                           kind="ExternalInput")
    out_d = nc.dram_tensor("out", [BL, CTOTP], F32, kind="ExternalOutput")

    with tile.TileContext(nc) as tc:
        with (
            tc.tile_pool(name="sb", bufs=1) as sb,
            tc.tile_pool(name="ps_att", bufs=1, space="PSUM") as ps_att_pool,
            tc.tile_pool(name="ps_mm", bufs=2, space="PSUM") as ps_mm_pool,
            tc.tile_pool(name="ps_t", bufs=2, space="PSUM") as ps_t_pool,
        ):
            # ---- constants / small inputs ----
            # aux packs sel [128,16] and eye4 (rows 0-3 of cols 16:20)
            aux = sb.tile([128, BL * BL + BL], F32R)
            nc.sync.dma_start(aux[:], aux_d[:])
            sel = aux[:, 0:BL * BL]
            eye4 = aux[0:BL, BL * BL:BL * BL + BL]
            bfc_sb = sb.tile([1, 4, FC], F32R)
            blc_sb = sb.tile([1, CTOTP + BL], F32R)
            if use_bias:
                nc.sync.dma_start(bfc_sb[:], bfc_d[:])
                nc.sync.dma_start(blc_sb[:], blc_d[:])
            ones_row = blc_sb[:, CTOTP:CTOTP + BL]


            # liT: [128, kt(8), b(4)]  (local_input transposed, 2D x BL)
            liT = sb.tile([128, 8, BL], F32R)
            nc.sync.dma_start(
                liT[:, 0:4, :],
                poolT_d[:].rearrange("(t p) b -> p t b", p=128),
            )

            # ---- lstm shard + DVE rowsum (exact fp32 adds) ----
            x_sb = sb.tile([128, BL, 4, D], F32R)
            part = sb.tile([128, BL, D], F32R)
            for b in range(BL):
                nc.sync.dma_start(
                    x_sb[:, b], x_d[b].rearrange("(t p) d -> p t d", p=128)
                )
                nc.vector.tensor_add(part[:, b], x_sb[:, b, 0, :],
                                     x_sb[:, b, 1, :])
                nc.vector.tensor_add(part[:, b], part[:, b], x_sb[:, b, 2, :])
                nc.vector.tensor_add(part[:, b], part[:, b], x_sb[:, b, 3, :])

            # ---- weights, in consumption order ----
            wfc_sb = sb.tile([128, 4, 8, FC], F32R)
            wlc_sb = []
            for h in range(4):
                nc.sync.dma_start(
                    wfc_sb[:, h], wfc_d[h].rearrange("(t p) n -> p t n", p=128)
                )
                wt = sb.tile([128, 4, HIERP[h]], F32R, tag=f"wlc{h}")
                ch = CHUNKS3 if h == 3 else _chunks(HIERP[h], 512)
                for (c0, n) in ch:
                    nc.sync.dma_start(
                        wt[:, :, c0:c0 + n],
                        wlc_d[h][:, c0:c0 + n].rearrange(
                            "(t p) c -> p t c", p=128))
                wlc_sb.append(wt)

            # ---- partition reduce: att[b, :] = colsum(part[:, b]) ----
            psum_att = ps_att_pool.tile([BL, D], F32)
            for b in range(BL):
                nc.tensor.matmul(
                    psum_att[:], sel[:, BL * b:BL * (b + 1)], part[:, b],
                    start=(b == 0), stop=(b == 3),
                )
            att_sb = sb.tile([BL, D], F32R)
            nc.vector.tensor_copy(att_sb[:], psum_att[:])

            # transpose att [4, 512] -> liT[:, 4+c, :]
            for c in range(4):
                pt = ps_t_pool.tile([128, BL], F32R, tag="pt")
                nc.tensor.transpose(pt[:], att_sb[:, 128 * c:128 * (c + 1)],
                                    eye4)
                nc.vector.tensor_copy(liT[:, 4 + c, :], pt[:])

            out_sb = sb.tile([BL, CTOTP], F32)

            # ---- per-level MLP ----
            off = 0
            for h in range(4):
                Ch = HIERP[h]
                # fc = relu(li @ w_fc[h] + b_fc[h])  -> psum [4, 512]
                psum_fc = ps_mm_pool.tile([BL, FC], F32, tag="fc")
                for kt in range(8):
                    nc.tensor.matmul(psum_fc[:], liT[:, kt, :],
                                     wfc_sb[:, h, kt, :],
                                     start=(kt == 0),
                                     stop=(not use_bias and kt == 7))
                if use_bias:
                    nc.tensor.matmul(psum_fc[:], ones_row, bfc_sb[:, h, :],
                                     start=False, stop=True)
                fc_sb = sb.tile([BL, FC], F32R, tag="fc_sb")
                nc.scalar.activation(fc_sb[:], psum_fc[:], AF.Relu)

                # fcT [128, kt(4), b]
                fcT = sb.tile([128, 4, BL], F32R, tag="fcT")
                for c in range(4):
                    pt = ps_t_pool.tile([128, BL], F32R, tag="pt")
                    nc.tensor.transpose(pt[:], fc_sb[:, 128 * c:128 * (c + 1)],
                                        eye4)
                    nc.vector.tensor_copy(fcT[:, c, :], pt[:])

                # scores = sigmoid(fc @ w_lc[h] + b_lc[h])
                score_chunks = CHUNKS3 if h == 3 else _chunks(Ch, 512)
                for (c0, n) in score_chunks:
                    ps = ps_mm_pool.tile([BL, 512], F32, tag="ps")
                    for kt in range(4):
                        nc.tensor.matmul(ps[:, :n], fcT[:, kt, :],
                                         wlc_sb[h][:, kt, c0:c0 + n],
                                         start=(kt == 0),
                                         stop=(not use_bias and kt == 3))
                    if use_bias:
                        nc.tensor.matmul(ps[:, :n], ones_row,
                                         blc_sb[:, off + c0:off + c0 + n],
                                         start=False, stop=True)
                    nc.scalar.activation(out_sb[:, off + c0:off + c0 + n],
                                         ps[:, :n], AF.Sigmoid)
                off += Ch

            nc.sync.dma_start(out_d[:, :CTOTP - 64],
                              out_sb[:, :CTOTP - 64])
            nc.sync.dma_start(out_d[:, CTOTP - 64:],
                              out_sb[:, CTOTP - 64:])

    nc.compile()
    return nc


def kernel(lstm_out, lstm_pool, w_at1, w_at2, w_fc, b_fc, w_lc, b_lc):
    global _NC, LAST_RESULTS
    use_bias = any(np.any(np.asarray(b) != 0) for b in list(b_fc) + list(b_lc))
    if use_bias not in _NC:
        _NC[use_bias] = _build(use_bias)
    nc = _NC[use_bias]

    lstm_out = np.ascontiguousarray(np.asarray(lstm_out, dtype=np.float32))
    lstm_pool = np.asarray(lstm_pool, dtype=np.float32)
    wfc_np = np.ascontiguousarray(np.stack([np.asarray(w, np.float32) for w in w_fc]))
    bfc_np = np.ascontiguousarray(np.stack([np.asarray(b, np.float32) for b in b_fc])[None])
    wlc_np = []
    blc_parts = []
    for h in range(4):
        wp = np.zeros([FC, HIERP[h]], np.float32)
        wp[:, :HIER[h]] = np.asarray(w_lc[h], np.float32)
        wlc_np.append(wp)
        bp = np.zeros([HIERP[h]], np.float32)
        bp[:HIER[h]] = np.asarray(b_lc[h], np.float32)
        blc_parts.append(bp)
    blc_np = np.ascontiguousarray(np.concatenate(blc_parts)[None])
    aux = np.zeros([128, BL * BL + BL], np.float32)
    for b in range(BL):
        aux[:, BL * b + b] = 1.0
    aux[0:BL, BL * BL:BL * BL + BL] = np.eye(BL, dtype=np.float32)
    blc_np = np.ascontiguousarray(
        np.concatenate([blc_np[0], np.ones([BL], np.float32)])[None])

    in_maps = []
    for k in range(NCORES):
        m = {
            "x": np.ascontiguousarray(lstm_out[BL * k: BL * (k + 1)]),
            "poolT": np.ascontiguousarray(lstm_pool[BL * k: BL * (k + 1)].T),
            "wfc": wfc_np,
            "bfc": bfc_np,
            "blc": blc_np,
            "aux": aux,
        }
        for h in range(4):
            m[f"wlc{h}"] = wlc_np[h]
        in_maps.append(m)

    expected = set()
    for alloc in nc.m.functions[0].allocations:
        if isinstance(alloc, mybir.MemoryLocationSet) and \
                alloc.kind == "ExternalInput":
            expected.add(alloc.memorylocations[0].name)
    in_maps = [{k: v for k, v in m.items() if k in expected}
               for m in in_maps]

    res = run_bass_kernel_spmd(nc, in_maps, list(range(NCORES)))
    LAST_RESULTS = res
    full = np.concatenate([r["out"] for r in res.results], axis=0)
    # trim per-level fp32r padding columns
    outs = []
    off = 0
    for h in range(4):
        outs.append(full[:, off:off + HIER[h]])
        off += HIERP[h]
    return np.ascontiguousarray(np.concatenate(outs, axis=1))
